# revision 12
# baseline (speedup 1.0000x reference)
"""
Trainium2 Bass kernel for nn_MetaAttention.

Computation (per batch b):
    rowsum[h,i]     = sum_j m[b,h,i,j]
    aggregated[i,j] = sum_h rowsum[h,i] * m[b,h,i,j]
    out[b]          = softmax(aggregated.flatten()).reshape(N, N)

Sharding: pure data parallel over B=16 across 8 cores (2 batches/core).

Memory-regime kernel: ~64 MB HBM traffic/core. The key constraint found
by profiling: SDMA per-engine rate collapses to ~16 GB/s when SBUF tiles
use only 112 of 128 partitions (the 784 = 112*7 row split), while
128-partition tiles stream at the full ~27 GB/s/engine (370-400 GB/s
aggregate over the two HWDGE rings). So the layout here is:

  - partition p holds rows 6p..6p+5 ("slot" t=0..5) of each 784x784
    image -> [128, 6, 784] main loads, 18816 B contiguous per partition,
    alternating between the sync and scalar HWDGE queues.
  - leftover rows 768..783 ride as a [16, 784] "extra" tile on
    partitions 112..127 (slot 6 of the same SBUF tile).

Per (b,h): rowsums on DVE (slots 0-2, extra) + ACT accum (slots 3-5);
scale-accumulate per slot: 0-2 on PE (diag matmul, PSUM-accumulated over
heads), 3 on DVE STT, 4-5 + extra on ACT-mult + GPSIMD-add. On the last
head the per-slot softmax max is fused into the final op
(tensor_tensor_reduce for the PSUM merges, gpsimd reduce for the rest).

Softmax: batch 0 resolves cross-partition max/sum via gpsimd
partition_all_reduce (keeps the PE queue clean) and is emitted
interleaved into batch 1's head loop so the load stream never stalls;
batch 1 uses the low-latency PE transpose/ones-matmul path and stores on
the idle HWDGE queues to shorten the tail.
"""

import numpy as np

B, H, N = 16, 12, 784
NCORES = 8
BPC = B // NCORES          # batches per core
P = 128                    # partitions
S = 6                      # main row-slots per partition (rows 0..767)
NME = P * S                # 768 main rows
XPB = 96                   # extra rows 768..783 live on partitions 96..111
XPE = XPB + 16             # (engine APs require start partition in {0,32,64,96})
JSPLITS = [(0, 512), (512, 272)]  # matmul free-dim splits (PSUM bank aligned)
PE_SLOTS = (0, 1, 2)

LAST_RESULT = None  # BassKernelResults of the most recent kernel() call


def build_program():
    import concourse.bacc as bacc
    import concourse.tile as tile
    from concourse import mybir
    from concourse import bass_isa

    f32 = mybir.dt.float32
    AX = mybir.AxisListType.X
    ADD = mybir.AluOpType.add
    MULT = mybir.AluOpType.mult
    MAXOP = mybir.AluOpType.max
    COPYF = mybir.ActivationFunctionType.Copy
    EXPF = mybir.ActivationFunctionType.Exp

    nc = bacc.Bacc("TRN2")
    x = nc.dram_tensor("x", [BPC, H, N, N], f32, kind="ExternalInput")
    ident = nc.dram_tensor("ident", [P, P], f32, kind="ExternalInput")
    y = nc.dram_tensor("y", [BPC, N, N], f32, kind="ExternalOutput")

    with tile.TileContext(nc) as tc:
        with (
            tc.tile_pool(name="mh", bufs=4) as mh_pool,
            tc.tile_pool(name="agg", bufs=2) as agg_pool,
            tc.tile_pool(name="acc", bufs=3, space="PSUM") as acc_pool,
            tc.tile_pool(name="psm", bufs=2, space="PSUM") as psm_pool,
            tc.tile_pool(name="dg", bufs=4) as dg_pool,
            tc.tile_pool(name="scr", bufs=4) as scr_pool,
            tc.tile_pool(name="sc2", bufs=6) as sc2_pool,
            tc.tile_pool(name="small", bufs=6) as small_pool,
            tc.tile_pool(name="consts", bufs=1) as const_pool,
        ):
            ident_sb = const_pool.tile([P, P], f32)
            nc.sync.dma_start(out=ident_sb, in_=ident[:, :])
            ones_sb = const_pool.tile([P, P], f32)
            nc.vector.memset(ones_sb, 1.0)

            state = {}

            def emit_head(b, h):
                st = state[b]
                agg, maxs, accs = st["agg"], st["maxs"], st["accs"]
                gi = b * H + h
                qm, qx = (nc.sync, nc.scalar) if gi % 2 == 0 else (nc.scalar, nc.sync)
                mh = mh_pool.tile([P, S + 1, N], f32, tag="mh")
                qm.dma_start(
                    out=mh[:, 0:S, :],
                    in_=x[b, h, 0:NME, :].rearrange("(p t) j -> p t j", p=P),
                )
                qx.dma_start(out=mh[XPB:XPE, S, :], in_=x[b, h, NME:N, :])

                rs = small_pool.tile([P, 8], f32, tag="rs")
                nc.vector.tensor_reduce(
                    out=rs[:, 0:3], in_=mh[:, 0:3, :], axis=AX, op=ADD
                )
                for s in (3, 4, 5):
                    scr = scr_pool.tile([P, N], f32, tag="scr")
                    nc.scalar.activation(
                        out=scr, in_=mh[:, s, :], func=COPYF, bias=0.0,
                        scale=1.0, accum_out=rs[:, s : s + 1],
                    )
                nc.vector.tensor_reduce(
                    out=rs[XPB:XPE, 6:7], in_=mh[XPB:XPE, S, :], axis=AX, op=ADD
                )

                last = h == H - 1
                # slots 0-2: PE diag matmul, PSUM accumulate over heads
                for s in PE_SLOTS:
                    dgt = dg_pool.tile([P, P], f32, tag="dg")
                    nc.vector.tensor_scalar_mul(
                        out=dgt, in0=ident_sb, scalar1=rs[:, s : s + 1]
                    )
                    for j0, jn in JSPLITS:
                        nc.tensor.matmul(
                            accs[s][:, j0 : j0 + jn],
                            lhsT=dgt,
                            rhs=mh[:, s, j0 : j0 + jn],
                            start=(h == 0),
                            stop=last,
                        )
                    if last:
                        # merge PSUM -> agg on ACT, then per-slot max on DVE
                        # (tensor_tensor_reduce is a custom DVE uop that
                        # wedges the exec unit on HW -- do not use it)
                        nc.scalar.activation(
                            out=agg[:, s, :], in_=accs[s][:, 0:N],
                            func=COPYF, bias=0.0, scale=1.0,
                        )
                        nc.vector.tensor_reduce(
                            out=maxs[:, s : s + 1], in_=agg[:, s, :],
                            axis=AX, op=MAXOP,
                        )
                # slot 3: DVE scalar_tensor_tensor chain
                if h == 0:
                    nc.vector.tensor_scalar_mul(
                        out=agg[:, 3, :], in0=mh[:, 3, :], scalar1=rs[:, 3:4]
                    )
                elif not last:
                    nc.vector.scalar_tensor_tensor(
                        out=agg[:, 3, :], in0=mh[:, 3, :], scalar=rs[:, 3:4],
                        in1=agg[:, 3, :], op0=MULT, op1=ADD,
                    )
                else:
                    sc2 = sc2_pool.tile([P, N], f32, tag="sc2")
                    nc.scalar.activation(
                        out=sc2, in_=mh[:, 3, :], func=COPYF, bias=0.0,
                        scale=rs[:, 3:4],
                    )
                    nc.gpsimd.tensor_tensor(
                        out=agg[:, 3, :], in0=sc2, in1=agg[:, 3, :], op=ADD
                    )
                    nc.vector.tensor_reduce(
                        out=maxs[:, 3:4], in_=agg[:, 3, :], axis=AX, op=MAXOP
                    )
                # slots 4, 5: ACT mult + GPSIMD add (DVE TTR on the last head
                # to fuse in the per-slot max)
                for s in (4, 5):
                    if h == 0:
                        nc.scalar.activation(
                            out=agg[:, s, :], in_=mh[:, s, :], func=COPYF,
                            bias=0.0, scale=rs[:, s : s + 1],
                        )
                    else:
                        sc2 = sc2_pool.tile([P, N], f32, tag="sc2")
                        nc.scalar.activation(
                            out=sc2, in_=mh[:, s, :], func=COPYF, bias=0.0,
                            scale=rs[:, s : s + 1],
                        )
                        nc.gpsimd.tensor_tensor(
                            out=agg[:, s, :], in0=sc2, in1=agg[:, s, :],
                            op=ADD,
                        )
                        if last:
                            nc.vector.tensor_reduce(
                                out=maxs[:, s : s + 1], in_=agg[:, s, :],
                                axis=AX, op=MAXOP,
                            )
                # extra rows 768..783 (slot S on partitions XPB..P)
                if h == 0:
                    nc.scalar.activation(
                        out=agg[XPB:XPE, S, :], in_=mh[XPB:XPE, S, :], func=COPYF,
                        bias=0.0, scale=rs[XPB:XPE, 6:7],
                    )
                else:
                    sc2 = sc2_pool.tile([P, N], f32, tag="sc2")
                    nc.scalar.activation(
                        out=sc2[XPB:XPE, :], in_=mh[XPB:XPE, S, :], func=COPYF,
                        bias=0.0, scale=rs[XPB:XPE, 6:7],
                    )
                    nc.gpsimd.tensor_tensor(
                        out=agg[XPB:XPE, S, :], in0=sc2[XPB:XPE, :],
                        in1=agg[XPB:XPE, S, :], op=ADD,
                    )
                    if last:
                        # (tensor_tensor_reduce needs partition base 0, so
                        # the extras' max is a separate reduce)
                        nc.vector.tensor_reduce(
                            out=maxs[XPB:XPE, 6:7], in_=agg[XPB:XPE, S, :],
                            axis=AX, op=MAXOP,
                        )

            def emit_softmax_stage(b, stg):
                st = state[b]
                agg, maxs, sums = st["agg"], st["maxs"], st["sums"]
                mode_pe = True  # partition_all_reduce path hangs on HW; PE path for both batches
                if stg == 0:
                    m1 = small_pool.tile([P, 1], f32, tag="m1")
                    nc.vector.tensor_reduce(
                        out=m1, in_=maxs[:, 0:7], axis=AX, op=MAXOP
                    )
                    negmax = small_pool.tile([P, 1], f32, tag="negmax")
                    if mode_pe:
                        tps = psm_pool.tile([1, P], f32, tag="ps", name=f"tps{b}")
                        nc.tensor.transpose(tps, m1, ident_sb)
                        gm = small_pool.tile([1, 1], f32, tag="gm")
                        nc.vector.tensor_reduce(
                            out=gm, in_=tps, axis=AX, op=MAXOP
                        )
                        bps = psm_pool.tile([P, 1], f32, tag="ps", name=f"bps{b}")
                        nc.tensor.matmul(
                            bps, lhsT=ones_sb[0:1, :], rhs=gm, start=True, stop=True
                        )
                        nc.scalar.mul(out=negmax, in_=bps, mul=-1.0)
                    else:
                        gmax = small_pool.tile([P, 1], f32, tag="gmax")
                        nc.gpsimd.partition_all_reduce(
                            gmax, m1, P, bass_isa.ReduceOp.max
                        )
                        nc.scalar.mul(out=negmax, in_=gmax, mul=-1.0)
                    st["negmax"] = negmax
                elif stg in (1, 2):
                    negmax = st["negmax"]
                    for s in (0, 1, 2) if stg == 1 else (3, 4, 5):
                        nc.scalar.activation(
                            out=agg[:, s, :], in_=agg[:, s, :], func=EXPF,
                            bias=negmax, scale=1.0,
                            accum_out=sums[:, s : s + 1],
                        )
                    if stg == 2:
                        nc.scalar.activation(
                            out=agg[XPB:XPE, S, :], in_=agg[XPB:XPE, S, :],
                            func=EXPF, bias=negmax[XPB:XPE, :], scale=1.0,
                            accum_out=sums[XPB:XPE, 6:7],
                        )
                elif stg == 3:
                    s1 = small_pool.tile([P, 1], f32, tag="s1")
                    nc.vector.tensor_reduce(
                        out=s1, in_=sums[:, 0:7], axis=AX, op=ADD
                    )
                    rinv = small_pool.tile([P, 1], f32, tag="rinv")
                    if mode_pe:
                        sps = psm_pool.tile([P, 1], f32, tag="ps", name=f"sps{b}")
                        nc.tensor.matmul(
                            sps, lhsT=ones_sb, rhs=s1, start=True, stop=True
                        )
                        nc.vector.reciprocal(out=rinv, in_=sps)
                    else:
                        gsum = small_pool.tile([P, 1], f32, tag="gsum")
                        nc.gpsimd.partition_all_reduce(
                            gsum, s1, P, bass_isa.ReduceOp.add
                        )
                        nc.vector.reciprocal(out=rinv, in_=gsum)
                    st["rinv"] = rinv
                elif stg in (4, 5, 6):
                    s0 = (stg - 4) * 2
                    rinv = st["rinv"]
                    nc.scalar.activation(
                        out=agg[:, s0, :], in_=agg[:, s0, :], func=COPYF,
                        bias=0.0, scale=rinv,
                    )
                    nc.vector.tensor_scalar_mul(
                        out=agg[:, s0 + 1, :], in0=agg[:, s0 + 1, :], scalar1=rinv
                    )
                    dst = y[b, 0:NME, :].rearrange("(p t) j -> p t j", p=P)
                    if b == 0:
                        eng = nc.gpsimd
                    else:
                        eng = nc.sync if stg % 2 == 0 else nc.scalar
                    eng.dma_start(
                        out=dst[:, s0 : s0 + 2, :], in_=agg[:, s0 : s0 + 2, :]
                    )
                else:  # stage 7: extra rows
                    rinv = st["rinv"]
                    nc.scalar.activation(
                        out=agg[XPB:XPE, S, :], in_=agg[XPB:XPE, S, :], func=COPYF,
                        bias=0.0, scale=rinv[XPB:XPE, :],
                    )
                    eng = nc.gpsimd if b == 0 else nc.sync
                    eng.dma_start(out=y[b, NME:N, :], in_=agg[XPB:XPE, S, :])

            # interleave batch 0's softmax into batch 1's head loop so the
            # DMA stream never pauses between batches
            STAGE_AT = {1: [0], 2: [1], 3: [2], 4: [3], 5: [4], 6: [5], 7: [6], 8: [7]}

            for b in range(BPC):
                agg = agg_pool.tile([P, S + 1, N], f32, tag="agg")
                maxs = small_pool.tile([P, 8], f32, tag="maxs")
                sums = small_pool.tile([P, 8], f32, tag="sums")
                nc.vector.memset(maxs, -1e30)
                nc.vector.memset(sums, 0.0)
                accs = [
                    acc_pool.tile([P, 1024], f32, tag="acc", name=f"acc_{b}_{s}")
                    for s in PE_SLOTS
                ]
                state[b] = dict(agg=agg, maxs=maxs, sums=sums, accs=accs)
                for h in range(H):
                    emit_head(b, h)
                    if b == 1:
                        for stg in STAGE_AT.get(h, []):
                            emit_softmax_stage(0, stg)
            for stg in range(8):
                emit_softmax_stage(BPC - 1, stg)

    nc.finalize()
    return nc


def kernel(mha_masks) -> np.ndarray:
    global LAST_RESULT
    from concourse.bass_utils import run_bass_kernel_spmd

    xfull = np.ascontiguousarray(np.asarray(mha_masks, dtype=np.float32))
    assert xfull.shape == (B, H, N, N), xfull.shape

    nc = build_program()
    ident = np.eye(P, dtype=np.float32)
    in_maps = [
        {"x": xfull[i * BPC : (i + 1) * BPC], "ident": ident}
        for i in range(NCORES)
    ]
    import os

    kw = {}
    if os.environ.get("KERNEL_TRACE_DIR"):
        kw = dict(trace=True, tmpdir=os.environ["KERNEL_TRACE_DIR"])
    res = run_bass_kernel_spmd(nc, in_maps, core_ids=list(range(NCORES)), **kw)
    LAST_RESULT = res
    out = np.concatenate(
        [np.asarray(r["y"], dtype=np.float32) for r in res.results], axis=0
    )
    return out


# revision 17
# speedup vs baseline: 1.0748x; 1.0748x over previous
"""
Trainium2 Bass kernel for nn_MetaAttention.

Computation (per batch b):
    rowsum[h,i]     = sum_j m[b,h,i,j]
    aggregated[i,j] = sum_h rowsum[h,i] * m[b,h,i,j]
    out[b]          = softmax(aggregated.flatten()).reshape(N, N)

Sharding: pure data parallel over B=16 across 8 cores (2 batches/core).

Memory-regime kernel, ~64 MB HBM traffic/core. Two profiling facts drive
the design:
  (1) SDMA per-engine rate collapses to ~16 GB/s for 112-partition tiles
      (784 = 112*7) but hits the full ~27 GB/s/engine (350-400 GB/s/core
      over the two HWDGE rings) for 128-partition tiles. So partition p
      holds rows 6p..6p+5 ("slots" 0-5) -> [128, 6, 784] main loads with
      18816 B contiguous per partition, alternating sync/scalar queues;
      leftover rows 768..783 ride as a [16, 784] "extra" (slot 6) on
      partitions 64..79; its dead lanes are NEVER read (the slot-6
      matmul contracts K=16 over partitions 64..79 only).
  (2) per-op fixed costs on the SBUF engines (~0.3-1 us each) are what
      starve the DMA, so the per-head op count is minimized:
      - ONE DVE multi-slot rowsum reduce (slots 0-3) + 3 ACT
        activation+accum rowsums (slots 4-6; the extras' dead lanes are
        pre-zeroed once so their rowsums are exactly 0)
      - ONE DVE broadcast-multiply builds the diag matrices of ALL 7
        slots: dg[p,s,i] = ident[p,i] * rs[p,s]
      - PE accumulates j in [0,512) of all 7 slots over heads into 7
        PSUM banks (diag matmul, 512-wide)
      - the j in [512,784) tail of all 7 slots is ONE GpSimd
        broadcast-mult + ONE GpSimd add into the SBUF agg
Softmax: batch 0 merges PSUM->SBUF with 7 ACT copies at its last head
(frees PSUM for batch 1 fast) and resolves cross-partition max/sum with
gpsimd partition_all_reduce, all staged inside batch 1's head loop so
the load stream never pauses. Batch 1 (the tail) skips the merge: exp
reads the PSUM accumulators directly (fused bias subtract), then uses
the low-latency PE transpose/ones-matmul reductions and stores on the
idle HWDGE queues.

NOTE: tensor_tensor_reduce (custom DVE uop) wedges the exec unit on real
HW - do not use it.
"""

import numpy as np

B, H, N = 16, 12, 784
NCORES = 8
BPC = B // NCORES          # batches per core
P = 128                    # partitions
S = 6                      # main row-slots per partition (rows 0..767)
NS = 7                     # total slots incl. the extras slot
NME = P * S                # 768 main rows
XPB = 64                   # extra rows 768..783 live on partitions 64..79
XPE = XPB + 16             # (base 64: legal for engine APs and PE lhsT)
JP = 512                   # PE covers j in [0,JP); SBUF engines the tail
JT = N - JP                # 272
MHBUFS = 5

LAST_RESULT = None  # BassKernelResults of the most recent kernel() call


def build_program():
    import concourse.bacc as bacc
    import concourse.tile as tile
    from concourse import mybir
    from concourse import bass_isa

    f32 = mybir.dt.float32
    AX = mybir.AxisListType.X
    ADD = mybir.AluOpType.add
    MULT = mybir.AluOpType.mult
    MAXOP = mybir.AluOpType.max
    COPYF = mybir.ActivationFunctionType.Copy
    EXPF = mybir.ActivationFunctionType.Exp

    nc = bacc.Bacc("TRN2")
    x = nc.dram_tensor("x", [BPC, H, N, N], f32, kind="ExternalInput")
    ident = nc.dram_tensor("ident", [P, P], f32, kind="ExternalInput")
    y = nc.dram_tensor("y", [BPC, N, N], f32, kind="ExternalOutput")

    with tile.TileContext(nc) as tc:
        with (
            tc.tile_pool(name="mh", bufs=MHBUFS) as mh_pool,
            tc.tile_pool(name="agg", bufs=2) as agg_pool,
            tc.tile_pool(name="acc", bufs=S, space="PSUM") as acc_pool,
            tc.tile_pool(name="acc6", bufs=1, space="PSUM") as acc6_pool,
            tc.tile_pool(name="dgp", bufs=2) as dg_pool,
            tc.tile_pool(name="sct", bufs=2) as sct_pool,
            tc.tile_pool(name="scr", bufs=3) as scr_pool,
            tc.tile_pool(name="small", bufs=6) as small_pool,
            tc.tile_pool(name="consts", bufs=1) as const_pool,
        ):
            ones_sb = const_pool.tile([P, P], f32)
            nc.vector.memset(ones_sb, 1.0)
            idst = const_pool.tile([P, NS, P], f32)
            for a in range(NS):
                nc.scalar.dma_start(out=idst[:, a, :], in_=ident[:, :])

            state = {}

            def emit_head(b, h):
                st = state[b]
                agg, maxs, accs = st["agg"], st["maxs"], st["accs"]
                gi = b * H + h
                qm, qx = (nc.sync, nc.scalar) if gi % 2 == 0 else (nc.scalar, nc.sync)
                mh = mh_pool.tile([P, NS, N], f32, tag="mh")
                qm.dma_start(
                    out=mh[:, 0:S, :],
                    in_=x[b, h, 0:NME, :].rearrange("(p t) j -> p t j", p=P),
                )
                qx.dma_start(out=mh[XPB:XPE, S, :], in_=x[b, h, NME:N, :])

                rs = small_pool.tile([P, 8], f32, tag="rs")
                nc.vector.memset(rs[:, 6:7], 0.0)
                nc.vector.tensor_reduce(
                    out=rs[:, 0:4], in_=mh[:, 0:4, :], axis=AX, op=ADD
                )
                for s in (4, 5):
                    scr = scr_pool.tile([P, N], f32, tag="scr")
                    nc.scalar.activation(
                        out=scr, in_=mh[:, s, :], func=COPYF, bias=0.0,
                        scale=1.0, accum_out=rs[:, s : s + 1],
                    )
                scr6 = scr_pool.tile([P, N], f32, tag="scr")
                nc.scalar.activation(
                    out=scr6[XPB:XPE, :], in_=mh[XPB:XPE, S, :], func=COPYF,
                    bias=0.0, scale=1.0, accum_out=rs[XPB:XPE, 6:7],
                )
                # diag matrices for all 7 slots in one DVE op
                dg = dg_pool.tile([P, NS, P], f32, tag="dg")
                rs_bc = rs[:, 0:NS].unsqueeze(2).broadcast_to([P, NS, P])
                nc.vector.tensor_tensor(out=dg, in0=idst, in1=rs_bc, op=MULT)
                # PE: j in [0,512) of slots 0-5, PSUM-accumulated over
                # heads; the extras slot contracts K=16 over partitions
                # 64..79 only (never touches its dead lanes) full-width
                last = h == H - 1
                acc6 = st["acc6"]
                for s in range(S):
                    nc.tensor.matmul(
                        accs[s][:, 0:JP],
                        lhsT=dg[:, s, :],
                        rhs=mh[:, s, 0:JP],
                        start=(h == 0),
                        stop=last,
                    )
                for j0, jn in ((0, JP), (JP, JT)):
                    nc.tensor.matmul(
                        acc6[:, j0 : j0 + jn],
                        lhsT=dg[XPB:XPE, S, :],
                        rhs=mh[XPB:XPE, S, j0 : j0 + jn],
                        start=(h == 0),
                        stop=last,
                    )
                # SBUF path: j in [512,784) of slots 0-5 in two GpSimd ops
                rs_bct = rs[:, 0:S].unsqueeze(2).broadcast_to([P, S, JT])
                if h == 0:
                    nc.gpsimd.tensor_tensor(
                        out=agg[:, 0:S, JP:N], in0=mh[:, 0:S, JP:N],
                        in1=rs_bct, op=MULT,
                    )
                else:
                    sct = sct_pool.tile([P, S, JT], f32, tag="sct")
                    nc.gpsimd.tensor_tensor(
                        out=sct, in0=mh[:, 0:S, JP:N], in1=rs_bct, op=MULT
                    )
                    nc.gpsimd.tensor_tensor(
                        out=agg[:, 0:S, JP:N], in0=sct, in1=agg[:, 0:S, JP:N],
                        op=ADD,
                    )
                if last and b == 0:
                    # merge PSUM->SBUF now so batch 1 can reuse the banks
                    for s in range(S):
                        nc.scalar.activation(
                            out=agg[:, s, 0:JP], in_=accs[s][:, 0:JP],
                            func=COPYF, bias=0.0, scale=1.0,
                        )
                    nc.scalar.activation(
                        out=agg[:, S, :], in_=acc6[:, 0:N],
                        func=COPYF, bias=0.0, scale=1.0,
                    )

            def emit_softmax_b0(stg):
                """Full-SBUF softmax for batch 0, staged into batch 1's
                head loop. Cross-partition reductions on gpsimd."""
                st = state[0]
                agg, maxs, sums = st["agg"], st["maxs"], st["sums"]
                if stg == 0:
                    for s in range(4):
                        nc.vector.tensor_reduce(
                            out=maxs[:, s : s + 1], in_=agg[:, s, :],
                            axis=AX, op=MAXOP,
                        )
                elif stg == 1:
                    for s in range(4, NS):
                        nc.vector.tensor_reduce(
                            out=maxs[:, s : s + 1], in_=agg[:, s, :],
                            axis=AX, op=MAXOP,
                        )
                elif stg == 2:
                    m1 = small_pool.tile([P, 1], f32, tag="m1")
                    nc.vector.tensor_reduce(
                        out=m1, in_=maxs[:, 0:NS], axis=AX, op=MAXOP
                    )
                    gmax = small_pool.tile([P, 1], f32, tag="gmax")
                    nc.gpsimd.partition_all_reduce(
                        gmax, m1, P, bass_isa.ReduceOp.max
                    )
                    negmax = small_pool.tile([P, 1], f32, tag="negmax")
                    nc.scalar.mul(out=negmax, in_=gmax, mul=-1.0)
                    st["negmax"] = negmax
                elif stg in (3, 4):
                    negmax = st["negmax"]
                    for s in (0, 1, 2) if stg == 3 else (3, 4, 5, 6):
                        nc.scalar.activation(
                            out=agg[:, s, :], in_=agg[:, s, :], func=EXPF,
                            bias=negmax, scale=1.0,
                            accum_out=sums[:, s : s + 1],
                        )
                elif stg == 5:
                    s1 = small_pool.tile([P, 1], f32, tag="s1")
                    nc.vector.tensor_reduce(
                        out=s1, in_=sums[:, 0:NS], axis=AX, op=ADD
                    )
                    gsum = small_pool.tile([P, 1], f32, tag="gsum")
                    nc.gpsimd.partition_all_reduce(
                        gsum, s1, P, bass_isa.ReduceOp.add
                    )
                    rinv = small_pool.tile([P, 1], f32, tag="rinv")
                    nc.vector.reciprocal(out=rinv, in_=gsum)
                    st["rinv"] = rinv
                elif stg in (6, 7, 8):
                    s0 = (stg - 6) * 2
                    rinv = st["rinv"]
                    nc.scalar.activation(
                        out=agg[:, s0, :], in_=agg[:, s0, :], func=COPYF,
                        bias=0.0, scale=rinv,
                    )
                    nc.vector.tensor_scalar_mul(
                        out=agg[:, s0 + 1, :], in0=agg[:, s0 + 1, :],
                        scalar1=rinv,
                    )
                    dst = y[0, 0:NME, :].rearrange("(p t) j -> p t j", p=P)
                    nc.gpsimd.dma_start(
                        out=dst[:, s0 : s0 + 2, :], in_=agg[:, s0 : s0 + 2, :]
                    )
                else:  # stage 9: extras
                    rinv = st["rinv"]
                    nc.scalar.activation(
                        out=agg[XPB:XPE, S, :], in_=agg[XPB:XPE, S, :],
                        func=COPYF, bias=0.0, scale=rinv[XPB:XPE, :],
                    )
                    nc.gpsimd.dma_start(
                        out=y[0, NME:N, :], in_=agg[XPB:XPE, S, :]
                    )

            def emit_tail_b1():
                """Batch 1 softmax: maxes straight off PSUM + SBUF tails,
                exp reads PSUM directly (no merge), gpsimd all_reduce for
                the cross-partition max, a PE ones-matmul (into a retired
                PSUM bank) for the sum, stores on the idle HWDGE queues."""
                st = state[1]
                agg, maxs, sums, accs, acc6 = (
                    st["agg"], st["maxs"], st["sums"], st["accs"], st["acc6"],
                )
                for s in range(S):
                    nc.vector.tensor_reduce(
                        out=maxs[:, s : s + 1], in_=accs[s][:, 0:JP],
                        axis=AX, op=MAXOP,
                    )
                nc.vector.tensor_reduce(
                    out=maxs[:, 6:7], in_=acc6[:, 0:N], axis=AX, op=MAXOP
                )
                mtail = small_pool.tile([P, 8], f32, tag="mtail")
                nc.vector.memset(mtail, -1e30)
                nc.vector.tensor_reduce(
                    out=mtail[:, 0:S], in_=agg[:, 0:S, JP:N], axis=AX, op=MAXOP
                )
                comb = small_pool.tile([P, 8], f32, tag="comb")
                nc.vector.tensor_tensor(
                    out=comb, in0=maxs, in1=mtail, op=MAXOP
                )
                m1 = small_pool.tile([P, 1], f32, tag="m1")
                nc.vector.tensor_reduce(
                    out=m1, in_=comb[:, 0:NS], axis=AX, op=MAXOP
                )
                gmax = small_pool.tile([P, 1], f32, tag="gmax")
                nc.gpsimd.partition_all_reduce(
                    gmax, m1, P, bass_isa.ReduceOp.max
                )
                negmax = small_pool.tile([P, 1], f32, tag="negmax")
                nc.scalar.mul(out=negmax, in_=gmax, mul=-1.0)
                # exp: mains straight from PSUM, slot-5..0 tails in one op
                for s in range(S):
                    nc.scalar.activation(
                        out=agg[:, s, 0:JP], in_=accs[s][:, 0:JP], func=EXPF,
                        bias=negmax, scale=1.0, accum_out=sums[:, s : s + 1],
                    )
                nc.scalar.activation(
                    out=agg[:, S, :], in_=acc6[:, 0:N], func=EXPF,
                    bias=negmax, scale=1.0, accum_out=sums[:, 6:7],
                )
                nc.scalar.activation(
                    out=agg[:, 0:S, JP:N], in_=agg[:, 0:S, JP:N], func=EXPF,
                    bias=negmax, scale=1.0, accum_out=sums[:, 7:8],
                )
                s1 = small_pool.tile([P, 1], f32, tag="s1")
                nc.vector.tensor_reduce(out=s1, in_=sums, axis=AX, op=ADD)
                # cross-partition sum + broadcast via ones-matmul into a
                # PSUM bank whose exp already retired
                sps = acc_pool.tile([P, JP], f32, tag="acc", name="sps1")
                nc.tensor.matmul(
                    sps[:, 0:1], lhsT=ones_sb, rhs=s1, start=True, stop=True
                )
                rinv = small_pool.tile([P, 1], f32, tag="rinv")
                nc.vector.reciprocal(out=rinv, in_=sps[:, 0:1])
                dst = y[1, 0:NME, :].rearrange("(p t) j -> p t j", p=P)
                for pi in range(3):
                    s0 = pi * 2
                    nc.scalar.activation(
                        out=agg[:, s0, :], in_=agg[:, s0, :], func=COPYF,
                        bias=0.0, scale=rinv,
                    )
                    nc.vector.tensor_scalar_mul(
                        out=agg[:, s0 + 1, :], in0=agg[:, s0 + 1, :],
                        scalar1=rinv,
                    )
                    eng = nc.sync if pi % 2 == 0 else nc.scalar
                    eng.dma_start(
                        out=dst[:, s0 : s0 + 2, :], in_=agg[:, s0 : s0 + 2, :]
                    )
                nc.scalar.activation(
                    out=agg[XPB:XPE, S, :], in_=agg[XPB:XPE, S, :],
                    func=COPYF, bias=0.0, scale=rinv[XPB:XPE, :],
                )
                nc.sync.dma_start(out=y[1, NME:N, :], in_=agg[XPB:XPE, S, :])

            STAGE_AT = {0: [0], 1: [1], 2: [2], 3: [3], 4: [4], 5: [5],
                        6: [6], 7: [7], 8: [8], 9: [9]}

            for b in range(BPC):
                agg = agg_pool.tile([P, NS, N], f32, tag="agg")
                maxs = small_pool.tile([P, 8], f32, tag="maxs")
                sums = small_pool.tile([P, 8], f32, tag="sums")
                nc.vector.memset(maxs, -1e30)
                nc.vector.memset(sums, 0.0)
                accs = [
                    acc_pool.tile([P, JP], f32, tag="acc", name=f"acc_{b}_{s}")
                    for s in range(S)
                ]
                acc6 = acc6_pool.tile([P, 1024], f32, tag="acc6", name=f"acc6_{b}")
                state[b] = dict(
                    agg=agg, maxs=maxs, sums=sums, accs=accs, acc6=acc6
                )
                for h in range(H):
                    emit_head(b, h)
                    if b == 1:
                        for stg in STAGE_AT.get(h, []):
                            emit_softmax_b0(stg)
            emit_tail_b1()

    nc.finalize()
    return nc


def kernel(mha_masks) -> np.ndarray:
    global LAST_RESULT
    from concourse.bass_utils import run_bass_kernel_spmd

    xfull = np.ascontiguousarray(np.asarray(mha_masks, dtype=np.float32))
    assert xfull.shape == (B, H, N, N), xfull.shape

    nc = build_program()
    ident = np.eye(P, dtype=np.float32)
    in_maps = [
        {"x": xfull[i * BPC : (i + 1) * BPC], "ident": ident}
        for i in range(NCORES)
    ]
    import os

    kw = {}
    if os.environ.get("KERNEL_TRACE_DIR"):
        kw = dict(trace=True, tmpdir=os.environ["KERNEL_TRACE_DIR"])
    res = run_bass_kernel_spmd(nc, in_maps, core_ids=list(range(NCORES)), **kw)
    LAST_RESULT = res
    out = np.concatenate(
        [np.asarray(r["y"], dtype=np.float32) for r in res.results], axis=0
    )
    return out
